# revision 1
# baseline (speedup 1.0000x reference)
"""Trainium2 Bass kernel for nn_DeformKernelConv2d.

Math (per batch image; shapes below are per core after sharding):
  offsets:  off = conv3x3(x, offset_w) + offset_b          -> dy,dx per (k, pixel)
  coords:   yc_k = dy_k + by_k ; xc_k = dx_k + bx_k        (scope-kernel space)
  phi:      phi_y[k,i] = relu(1-|yc_k - i|), i=0..3        (likewise phi_x)
  Phi:      Phi_k[4*yi+xi] = phi_y[k,yi] * phi_x[k,xi]     (bilinear weights, 16 per k)
  samp:     samp_k[c] = sum_s wflat[c,s] * Phi_k[s]        (matmul over s=16)
  out:      out[c] = sum_k samp_k[c] * x_k[c]              (x_k = 3x3-shifted x)

Device mapping:
  - 8 cores: (batch b, H-half); each core does 28 rows with a 1-row halo.
  - The offset conv is fused with the affine (coord - grid line i) expansion
    into one 9-tap accumulated matmul producing T[72, pix] (rows (k, axis, i)),
    with per-row bias = offset_b + base - i folded into the Abs activation.
  - phi on ScalarE (Abs then Relu), phi->Y/X row replication by DMA,
    Phi products + final MAC on VectorE, samp matmuls row-packed 4x on PE.
"""

import numpy as np
import ml_dtypes

B, C, H, W = 4, 128, 56, 56
HC = H // 2            # 28 rows per core
NPIX = HC * W          # 1568
CH = 7                 # chunk height (rows)
NCH = HC // CH         # 4 chunks
CHN = CH * W           # 392 columns per chunk
RA, RB = 58, 60        # padded row lengths: xbfA data at col 1, xbfB at col 2

_BF16 = ml_dtypes.bfloat16
_cache = {}


def _build_program(repeat=1):
    import concourse.tile as tile
    import concourse.mybir as mybir
    from concourse import bacc

    fp32 = mybir.dt.float32
    bf16 = mybir.dt.bfloat16
    AF = mybir.ActivationFunctionType

    nc = bacc.Bacc("TRN2", target_bir_lowering=False, debug=False, num_devices=8)
    xs_d = nc.dram_tensor("xs", [C, HC + 2, W], fp32, kind="ExternalInput")
    lhsT_d = nc.dram_tensor("lhsT", [C, 9 * 72], bf16, kind="ExternalInput")
    w4T_d = nc.dram_tensor("w4T", [C, C], bf16, kind="ExternalInput")
    bias_d = nc.dram_tensor("bias72", [72, 1], fp32, kind="ExternalInput")
    out_d = nc.dram_tensor("out", [C, HC, W], fp32, kind="ExternalOutput")

    with tile.TileContext(nc) as tc:
        with (
            tc.tile_pool(name="const", bufs=1) as cp,
            tc.tile_pool(name="work", bufs=1) as wp,
            tc.tile_pool(name="tmp", bufs=3) as tp,
            tc.tile_pool(name="psT", bufs=2, space="PSUM") as ppT,
            tc.tile_pool(name="psS", bufs=5, space="PSUM") as ppS,
        ):
            lhsT = cp.tile([C, 9 * 72], bf16)
            nc.sync.dma_start(lhsT[:], lhsT_d[:])
            w4T = cp.tile([C, C], bf16)
            nc.sync.dma_start(w4T[:], w4T_d[:])
            bias = cp.tile([72, 1], fp32)
            nc.sync.dma_start(bias[:], bias_d[:])

            xbfA = cp.tile([C, HC + 2, RA], bf16)
            xbfB = cp.tile([C, HC + 2, RB], bf16)
            nc.vector.memset(xbfA[:, :, 0:1], 0)
            nc.vector.memset(xbfA[:, :, 57:58], 0)
            nc.vector.memset(xbfB[:, :, 0:2], 0)
            nc.vector.memset(xbfB[:, :, 58:60], 0)

            phi = wp.tile([72, NCH, CHN], bf16)
            Ya = wp.tile([C, NCH, CHN], bf16)
            Xa = wp.tile([C, NCH, CHN], bf16)
            Yb = wp.tile([C, NCH, CHN], bf16)
            Xb = wp.tile([C, NCH, CHN], bf16)
            Yc = wp.tile([16, NCH, CHN], bf16)
            Xc = wp.tile([16, NCH, CHN], bf16)
            PhA = wp.tile([C, NCH, CHN], bf16)
            PhB = wp.tile([C, NCH, CHN], bf16)
            PhC = wp.tile([16, NCH, CHN], bf16)
            samp = wp.tile([C, 9, NPIX], bf16)
            prod = wp.tile([C, 9, NPIX], bf16)
            t1 = wp.tile([C, 4, NPIX], bf16)
            t2 = wp.tile([C, 2, NPIX], bf16)
            t3 = wp.tile([C, NPIX], bf16)
            res = wp.tile([C, NPIX], fp32)

            # replication views (single strided partition dim per DMA side —
            # multi-dim partition APs mislower in the DMA path)
            phiV = phi[:].rearrange("(k a i) c n -> k a i c n", k=9, a=2, i=4)

            def rep_view(t):
                return t[:].rearrange("(g h y x) c n -> g h y x c n", h=2, y=4, x=4)

            YaV, XaV, YbV, XbV = rep_view(Ya), rep_view(Xa), rep_view(Yb), rep_view(Xb)
            YcV = Yc[:].rearrange("(y x) c n -> y x c n", x=4)
            XcV = Xc[:].rearrange("(y x) c n -> y x c n", x=4)

            for _rep in range(repeat):
                nc.gpsimd.dma_start(xbfA[:, :, 1:57], xs_d[:])  # casts fp32->bf16
                nc.gpsimd.dma_start(xbfB[:, :, 2:58], xs_d[:])

                # ---- T matmul (offset conv + affine expansion) + phi ----
                for ch in range(NCH):
                    psT = ppT.tile([72, CHN], fp32, tag="psT")
                    for tap in range(9):
                        di, dj = tap // 3, tap % 3
                        rhs = xbfA[:, ch * CH + di : ch * CH + di + CH, dj : dj + W]
                        nc.tensor.matmul(
                            psT[:],
                            lhsT[:, tap * 72 : (tap + 1) * 72],
                            rhs,
                            start=(tap == 0),
                            stop=(tap == 8),
                        )
                    u = tp.tile([72, CHN], fp32, tag="u")
                    nc.scalar.activation(u[:], psT[:], AF.Abs, bias=bias[:], scale=1.0)
                    nc.scalar.activation(
                        phi[:, ch, :], u[:], AF.Relu, bias=1.0, scale=-1.0
                    )

                # ---- replicate phi rows into Y/X s-patterns (72 DMAs) ----
                for g in range(4):
                    for xi in range(4):
                        nc.sync.dma_start(YaV[g, 0, :, xi], phiV[g, 0, :])
                        nc.sync.dma_start(YbV[g, 0, :, xi], phiV[4 + g, 0, :])
                    for yi in range(4):
                        nc.sync.dma_start(XaV[g, 0, yi, :], phiV[g, 1, :])
                        nc.sync.dma_start(XbV[g, 0, yi, :], phiV[4 + g, 1, :])
                for xi in range(4):
                    nc.sync.dma_start(YcV[:, xi], phiV[8, 0, :])
                for yi in range(4):
                    nc.sync.dma_start(XcV[yi, :], phiV[8, 1, :])

                # ---- Phi products, samp matmuls (row-packed), PSUM drain ----
                for ch in range(NCH):
                    nc.vector.tensor_mul(PhA[:, ch, :], Ya[:, ch, :], Xa[:, ch, :])
                    nc.vector.tensor_mul(PhB[:, ch, :], Yb[:, ch, :], Xb[:, ch, :])
                    nc.vector.tensor_mul(PhC[:, ch, :], Yc[:, ch, :], Xc[:, ch, :])
                    for k in range(9):
                        g = k % 4
                        src = (PhA, PhB, PhC)[k // 4]
                        base = 32 * g if k < 8 else 0
                        psS = ppS.tile([C, CHN], fp32, tag="psS")
                        nc.tensor.matmul(
                            psS[:],
                            w4T[base : base + 16, :],
                            src[base : base + 16, ch, :],
                            start=True,
                            stop=True,
                            tile_position=(base, 0),
                        )
                        dst = samp[:, k, ch * CHN : (ch + 1) * CHN]
                        if k < 6:
                            nc.scalar.copy(dst, psS[:])
                        else:
                            nc.vector.tensor_copy(out=dst, in_=psS[:])

                # ---- products with shifted x, tree sum, store ----
                for k in range(9):
                    di, dj = k // 3, k % 3
                    if dj == 1:
                        xsrc, coff = xbfB, dj + 1  # col offset 2: 4B aligned
                    else:
                        xsrc, coff = xbfA, dj  # col offsets 0, 2
                    xv = xsrc[:, di : di + HC, coff : coff + W]
                    nc.vector.tensor_mul(
                        prod[:, k, :].rearrange("p (h w) -> p h w", h=HC),
                        samp[:, k, :].rearrange("p (h w) -> p h w", h=HC),
                        xv,
                    )
                nc.vector.tensor_add(t1[:], prod[:, 0:4, :], prod[:, 4:8, :])
                nc.vector.tensor_add(t2[:], t1[:, 0:2, :], t1[:, 2:4, :])
                nc.vector.tensor_add(t3[:], t2[:, 0, :], t2[:, 1, :])
                nc.vector.tensor_add(res[:], t3[:], prod[:, 8, :])
                nc.sync.dma_start(
                    out_d[:], res[:].rearrange("p (h w) -> p h w", h=HC)
                )

    nc.finalize()
    return nc


def _prep_inputs(x, offset_w, offset_b, weight):
    """Host-side sharding + weight reshaping. Returns per-core input maps."""
    x = np.asarray(x, dtype=np.float32)
    offset_w = np.asarray(offset_w, dtype=np.float32)
    offset_b = np.asarray(offset_b, dtype=np.float32)
    weight = np.asarray(weight, dtype=np.float32)

    # lhsT[c, tap*72 + k*8 + axis*4 + i] = offset_w[2k+axis, c, tap//3, tap%3]
    ow = offset_w.reshape(9, 2, C, 3, 3)  # [k, axis, c, di, dj]
    lhsT = np.transpose(ow, (2, 3, 4, 0, 1))  # [c, di, dj, k, axis]
    lhsT = np.repeat(lhsT[..., None], 4, axis=-1)  # [c, di, dj, k, axis, i]
    lhsT = np.ascontiguousarray(lhsT.reshape(C, 648)).astype(_BF16)

    # w4T rows 32g+s = weight[:, s//4, s%4]
    w4T = np.zeros((C, C), dtype=_BF16)
    wT = weight.reshape(C, 16).T.astype(_BF16)  # [16, C]
    for g in range(4):
        w4T[32 * g : 32 * g + 16, :] = wT

    # bias72[k*8+axis*4+i] = offset_b[2k+axis] + base - i
    base = np.arange(3, dtype=np.float32) + 0.5
    bias = np.zeros((9, 2, 4), dtype=np.float32)
    for k in range(9):
        for axis in range(2):
            bv = base[k // 3] if axis == 0 else base[k % 3]
            bias[k, axis, :] = offset_b[2 * k + axis] + bv - np.arange(4)
    bias72 = bias.reshape(72, 1)

    in_maps = []
    for core in range(8):
        b, half = core // 2, core % 2
        h0 = half * HC
        xs = np.zeros((C, HC + 2, W), dtype=np.float32)
        lo, hi = h0 - 1, h0 + HC + 1
        slo, shi = max(lo, 0), min(hi, H)
        xs[:, slo - lo : slo - lo + (shi - slo), :] = x[b, :, slo:shi, :]
        in_maps.append({"xs": xs, "lhsT": lhsT, "w4T": w4T, "bias72": bias72})
    return in_maps


def kernel(x, offset_w, offset_b, weight):
    from concourse.bass_utils import run_bass_kernel_spmd

    if "nc" not in _cache:
        _cache["nc"] = _build_program()
    nc = _cache["nc"]

    in_maps = _prep_inputs(x, offset_w, offset_b, weight)
    res = run_bass_kernel_spmd(nc, in_maps, core_ids=list(range(8)))

    out = np.zeros((B, C, H, W), dtype=np.float32)
    for core in range(8):
        b, half = core // 2, core % 2
        out[b, :, half * HC : (half + 1) * HC, :] = res.results[core]["out"].reshape(
            C, HC, W
        )
    return out



# revision 3
# speedup vs baseline: 1.4314x; 1.4314x over previous
"""Trainium2 Bass kernel for nn_DeformKernelConv2d (v2: log-space replication).

Math (per batch image; shapes below are per core after sharding):
  offsets:  off = conv3x3(x, offset_w) + offset_b          -> dy,dx per (k, pixel)
  coords:   yc_k = dy_k + by_k ; xc_k = dx_k + bx_k        (scope-kernel space)
  phi:      phi_y[k,i] = relu(1-|yc_k - i|), i=0..3        (likewise phi_x)
  Phi:      Phi_k[4*yi+xi] = phi_y[k,yi] * phi_x[k,xi]     (bilinear weights, 16 per k)
  samp:     samp_k[c] = sum_s wflat[c,s] * Phi_k[s]        (matmul over s=16)
  out:      out[c] = sum_k samp_k[c] * x_k[c]              (x_k = 3x3-shifted x)

v2 pipeline (all per core; 4 chunks of 7 rows = 392 px):
  1. T matmul: 9-tap accumulated matmul -> psT[72, 392] (rows (k, axis, i))
  2. u = Abs(psT + bias)  [Scalar];  m = min(u, 1) [Vector];
     lnphi = Ln(1+eps - m) [Scalar]  (== ln(relu(1-u) + ~eps), bf16)
  3. Replication matmuls: for di in 0..2, psP = R_di^T @ lnphi: partition
     32*dj+4*yi+xi of psP holds ln(phi_y[k,yi]) + ln(phi_x[k,xi]) for
     k = 3*di+dj (PSUM accumulation does the product in log space).
  4. Ph_di = Exp(psP) [Scalar] -> SBUF bf16: the bilinear weights Phi_k.
  5. samp matmuls: per (di, dj): wT (16 rows at base 32*dj) @ Ph_di slice,
     row-group packed -> psS[128, 392] fp32.
  6. prod = psS * x_shift(di,dj): Vector fused from PSUM, or Scalar drain +
     Vector bf16 mul (split controlled by SDRAIN to balance engines).
  7. k-sum: 9 accumulating identity matmuls -> out_ps (KSUM="pe"), or a
     bf16 add tree on Vector (KSUM="vec").
  8. res = Copy(out_ps) [Scalar] -> DMA out.
"""

import numpy as np
import ml_dtypes

B, C, H, W = 4, 128, 56, 56
HC = H // 2            # 28 rows per core
NPIX = HC * W          # 1568
CH = 7                 # chunk height (rows)
NCH = HC // CH         # 4 chunks
CHN = CH * W           # 392 columns per chunk
RA, RB = 58, 60        # padded row lengths: xbfA data at col 1, xbfB at col 2
PSB = 512              # psum bank size in fp32 elems

KSUM = "pe"            # "pe": identity-matmul k-sum; "vec": vector add tree
# (chunk, di) groups whose samp drain goes scalar-drain + bf16-mul path
SDRAIN = {(0, 0), (1, 0), (2, 0), (3, 0)}

_BF16 = ml_dtypes.bfloat16
_cache = {}


def _x_window(xbfA, xbfB, ch, di, dj):
    """bf16 view of x shifted by tap (di, dj) for chunk ch: [C, CH, W]."""
    if dj == 1:
        xsrc, coff = xbfB, 2
    else:
        xsrc, coff = xbfA, dj
    r0 = ch * CH + di
    return xsrc[:, r0 : r0 + CH, coff : coff + W]


def _build_program():
    import concourse.tile as tile
    import concourse.mybir as mybir
    from concourse import bacc

    fp32 = mybir.dt.float32
    bf16 = mybir.dt.bfloat16
    AF = mybir.ActivationFunctionType

    nc = bacc.Bacc("TRN2", target_bir_lowering=False, debug=False, num_devices=8)
    xs_d = nc.dram_tensor("xs", [C, HC + 2, W], fp32, kind="ExternalInput")
    lhsT_d = nc.dram_tensor("lhsT", [C, 9 * 72], bf16, kind="ExternalInput")
    w4T_d = nc.dram_tensor("w4T", [C, C], bf16, kind="ExternalInput")
    R3_d = nc.dram_tensor("R3", [72, 3 * 80], bf16, kind="ExternalInput")
    idm_d = nc.dram_tensor("idm", [C, C], bf16, kind="ExternalInput")
    bias_d = nc.dram_tensor("bias72", [72, 1], fp32, kind="ExternalInput")
    out_d = nc.dram_tensor("out", [C, HC, W], fp32, kind="ExternalOutput")

    with tile.TileContext(nc) as tc:
        with (
            tc.tile_pool(name="const", bufs=1) as cp,
            tc.tile_pool(name="work", bufs=1) as wp,
            tc.tile_pool(name="tmp", bufs=3) as tp,
            tc.tile_pool(name="psT", bufs=1, space="PSUM") as ppT,
            tc.tile_pool(name="psP", bufs=3, space="PSUM") as ppP,
            tc.tile_pool(name="psS", bufs=3, space="PSUM") as ppS,
            tc.tile_pool(name="psO", bufs=1, space="PSUM") as ppO,
        ):
            lhsT = cp.tile([C, 9 * 72], bf16)
            nc.sync.dma_start(lhsT[:], lhsT_d[:])
            w4T = cp.tile([C, C], bf16)
            nc.sync.dma_start(w4T[:], w4T_d[:])
            R3 = cp.tile([72, 3 * 80], bf16)
            nc.sync.dma_start(R3[:], R3_d[:])
            idm = cp.tile([C, C], bf16)
            nc.sync.dma_start(idm[:], idm_d[:])
            bias = cp.tile([72, 1], fp32)
            nc.sync.dma_start(bias[:], bias_d[:])

            xbfA = cp.tile([C, HC + 2, RA], bf16)
            xbfB = cp.tile([C, HC + 2, RB], bf16)
            nc.vector.memset(xbfA[:, :, 0:1], 0)
            nc.vector.memset(xbfA[:, :, 57:58], 0)
            nc.vector.memset(xbfB[:, :, 0:2], 0)
            nc.vector.memset(xbfB[:, :, 58:60], 0)
            nc.gpsimd.dma_start(xbfA[:, :, 1:57], xs_d[:])  # casts fp32->bf16
            nc.gpsimd.dma_start(xbfB[:, :, 2:58], xs_d[:])

            for ch in range(NCH):
                # ---- T matmul (offset conv + affine expansion) ----
                psT = ppT.tile([72, CHN], fp32, tag="psT")
                for tap in range(9):
                    di, dj = tap // 3, tap % 3
                    rhs = xbfA[:, ch * CH + di : ch * CH + di + CH, dj : dj + W]
                    nc.tensor.matmul(
                        psT[:],
                        lhsT[:, tap * 72 : (tap + 1) * 72],
                        rhs,
                        start=(tap == 0),
                        stop=(tap == 8),
                    )

                # ---- phi in log space ----
                u = tp.tile([72, CHN], fp32, tag="u")
                nc.scalar.activation(u[:], psT[:], AF.Abs, bias=bias[:], scale=1.0)
                m = tp.tile([72, CHN], fp32, tag="m")
                nc.vector.tensor_scalar_min(m[:], u[:], 1.0 - 1e-6)
                lnphi = tp.tile([72, CHN], bf16, tag="lnphi")
                nc.scalar.activation(lnphi[:], m[:], AF.Ln, bias=1.0, scale=-1.0)

                prods = []
                for di in range(3):
                    # ---- replication matmul + exp -> Phi patterns ----
                    psP = ppP.tile([80, CHN], fp32, tag="psP")
                    nc.tensor.matmul(
                        psP[:],
                        R3[:, di * 80 : (di + 1) * 80],
                        lnphi[:],
                        start=True,
                        stop=True,
                    )
                    Ph = tp.tile([80, CHN], bf16, tag="Ph")
                    nc.scalar.activation(Ph[:], psP[:], AF.Exp)

                    # ---- samp matmuls (row-group packed) + drain/mul ----
                    prod = tp.tile([C, 3, CHN], bf16, tag="prod")
                    sdrain = (ch, di) in SDRAIN
                    psSs = []
                    for dj in range(3):
                        psS = ppS.tile([C, CHN], fp32, tag="psS")
                        nc.tensor.matmul(
                            psS[:],
                            w4T[32 * dj : 32 * dj + 16, :],
                            Ph[32 * dj : 32 * dj + 16, :],
                            start=True,
                            stop=True,
                            tile_position=(32 * dj, 0),
                        )
                        psSs.append(psS)
                    for dj in range(3):
                        xv = _x_window(xbfA, xbfB, ch, di, dj)
                        pv = prod[:, dj, :].rearrange("p (h w) -> p h w", h=CH)
                        sv = psSs[dj][:].rearrange("p (h w) -> p h w", h=CH)
                        if sdrain:
                            dr = tp.tile([C, CHN], bf16, tag="dr")
                            nc.scalar.copy(dr[:], psSs[dj][:])
                            nc.vector.tensor_mul(
                                pv, dr[:].rearrange("p (h w) -> p h w", h=CH), xv
                            )
                        else:
                            nc.vector.tensor_mul(pv, sv, xv)
                    prods.append(prod)

                # ---- k-sum ----
                res = tp.tile([C, CHN], fp32, tag="res")
                if KSUM == "pe":
                    psO = ppO.tile([C, CHN], fp32, tag="psO")
                    for di in range(3):
                        for dj in range(3):
                            nc.tensor.matmul(
                                psO[:],
                                idm[:],
                                prods[di][:, dj, :],
                                start=(di == 0 and dj == 0),
                                stop=(di == 2 and dj == 2),
                            )
                    nc.scalar.copy(res[:], psO[:])
                else:
                    t1 = tp.tile([C, 3, CHN], bf16, tag="t1")
                    nc.vector.tensor_add(t1[:], prods[0][:], prods[1][:])
                    nc.vector.tensor_add(t1[:], t1[:], prods[2][:])
                    t2 = tp.tile([C, CHN], bf16, tag="t2")
                    nc.vector.tensor_add(t2[:], t1[:, 0, :], t1[:, 1, :])
                    nc.vector.tensor_add(res[:], t2[:], t1[:, 2, :])
                nc.sync.dma_start(
                    out_d[:, ch * CH : (ch + 1) * CH, :],
                    res[:].rearrange("p (h w) -> p h w", h=CH),
                )

    nc.finalize()
    return nc


def _prep_inputs(x, offset_w, offset_b, weight):
    """Host-side sharding + weight reshaping. Returns per-core input maps."""
    x = np.asarray(x, dtype=np.float32)
    offset_w = np.asarray(offset_w, dtype=np.float32)
    offset_b = np.asarray(offset_b, dtype=np.float32)
    weight = np.asarray(weight, dtype=np.float32)

    # lhsT[c, tap*72 + k*8 + axis*4 + i] = offset_w[2k+axis, c, tap//3, tap%3]
    ow = offset_w.reshape(9, 2, C, 3, 3)  # [k, axis, c, di, dj]
    lhsT = np.transpose(ow, (2, 3, 4, 0, 1))  # [c, di, dj, k, axis]
    lhsT = np.repeat(lhsT[..., None], 4, axis=-1)  # [c, di, dj, k, axis, i]
    lhsT = np.ascontiguousarray(lhsT.reshape(C, 648)).astype(_BF16)

    # w4T rows 32g+s = weight[:, s//4, s%4]
    w4T = np.zeros((C, C), dtype=_BF16)
    wT = weight.reshape(C, 16).T.astype(_BF16)  # [16, C]
    for g in range(4):
        w4T[32 * g : 32 * g + 16, :] = wT

    # R3[8k+4a+i, 80*di + 32*dj + 4*yi + xi]: log-space replication matrices
    R3 = np.zeros((72, 240), dtype=_BF16)
    for di in range(3):
        for dj in range(3):
            k = 3 * di + dj
            for yi in range(4):
                for xi in range(4):
                    col = 80 * di + 32 * dj + 4 * yi + xi
                    R3[8 * k + yi, col] = 1
                    R3[8 * k + 4 + xi, col] = 1

    idm = np.eye(C, dtype=_BF16)

    # bias72[k*8+axis*4+i] = offset_b[2k+axis] + base - i
    base = np.arange(3, dtype=np.float32) + 0.5
    bias = np.zeros((9, 2, 4), dtype=np.float32)
    for k in range(9):
        for axis in range(2):
            bv = base[k // 3] if axis == 0 else base[k % 3]
            bias[k, axis, :] = offset_b[2 * k + axis] + bv - np.arange(4)
    bias72 = bias.reshape(72, 1)

    in_maps = []
    for core in range(8):
        b, half = core // 2, core % 2
        h0 = half * HC
        xs = np.zeros((C, HC + 2, W), dtype=np.float32)
        lo, hi = h0 - 1, h0 + HC + 1
        slo, shi = max(lo, 0), min(hi, H)
        xs[:, slo - lo : slo - lo + (shi - slo), :] = x[b, :, slo:shi, :]
        in_maps.append(
            {
                "xs": xs,
                "lhsT": lhsT,
                "w4T": w4T,
                "R3": R3,
                "idm": idm,
                "bias72": bias72,
            }
        )
    return in_maps


def kernel(x, offset_w, offset_b, weight):
    from concourse.bass_utils import run_bass_kernel_spmd

    if "nc" not in _cache:
        _cache["nc"] = _build_program()
    nc = _cache["nc"]

    in_maps = _prep_inputs(x, offset_w, offset_b, weight)
    res = run_bass_kernel_spmd(nc, in_maps, core_ids=list(range(8)))

    out = np.zeros((B, C, H, W), dtype=np.float32)
    for core in range(8):
        b, half = core // 2, core % 2
        out[b, :, half * HC : (half + 1) * HC, :] = res.results[core]["out"].reshape(
            C, HC, W
        )
    return out


# revision 4
# speedup vs baseline: 1.8404x; 1.2857x over previous
"""Trainium2 Bass kernel for nn_DeformKernelConv2d (v2: log-space replication).

Math (per batch image; shapes below are per core after sharding):
  offsets:  off = conv3x3(x, offset_w) + offset_b          -> dy,dx per (k, pixel)
  coords:   yc_k = dy_k + by_k ; xc_k = dx_k + bx_k        (scope-kernel space)
  phi:      phi_y[k,i] = relu(1-|yc_k - i|), i=0..3        (likewise phi_x)
  Phi:      Phi_k[4*yi+xi] = phi_y[k,yi] * phi_x[k,xi]     (bilinear weights, 16 per k)
  samp:     samp_k[c] = sum_s wflat[c,s] * Phi_k[s]        (matmul over s=16)
  out:      out[c] = sum_k samp_k[c] * x_k[c]              (x_k = 3x3-shifted x)

v2 pipeline (all per core; 4 chunks of 7 rows = 392 px):
  1. T matmul: 9-tap accumulated matmul -> psT[72, 392] (rows (k, axis, i))
  2. u = Abs(psT + bias)  [Scalar];  m = min(u, 1) [Vector];
     lnphi = Ln(1+eps - m) [Scalar]  (== ln(relu(1-u) + ~eps), bf16)
  3. Replication matmuls: for di in 0..2, psP = R_di^T @ lnphi: partition
     32*dj+4*yi+xi of psP holds ln(phi_y[k,yi]) + ln(phi_x[k,xi]) for
     k = 3*di+dj (PSUM accumulation does the product in log space).
  4. Ph_di = Exp(psP) [Scalar] -> SBUF bf16: the bilinear weights Phi_k.
  5. samp matmuls: per (di, dj): wT (16 rows at base 32*dj) @ Ph_di slice,
     row-group packed -> psS[128, 392] fp32.
  6. prod = psS * x_shift(di,dj): Vector fused from PSUM, or Scalar drain +
     Vector bf16 mul (split controlled by SDRAIN to balance engines).
  7. k-sum: 9 accumulating identity matmuls -> out_ps (KSUM="pe"), or a
     bf16 add tree on Vector (KSUM="vec").
  8. res = Copy(out_ps) [Scalar] -> DMA out.
"""

import numpy as np
import ml_dtypes

B, C, H, W = 4, 128, 56, 56
HC = H // 2            # 28 rows per core
NPIX = HC * W          # 1568
CH = 7                 # chunk height (rows)
NCH = HC // CH         # 4 chunks
CHN = CH * W           # 392 columns per chunk
RA, RB = 58, 60        # padded row lengths: xbfA data at col 1, xbfB at col 2
PSB = 512              # psum bank size in fp32 elems

KSUM = "pe"            # "pe": identity-matmul k-sum; "vec": vector add tree
# (chunk, di) groups whose samp drain goes scalar-drain + bf16-mul path
SDRAIN = {(0, 0), (1, 0), (2, 0), (3, 0)}

_BF16 = ml_dtypes.bfloat16
_cache = {}


def _x_window(xbfA, xbfB, ch, di, dj):
    """bf16 view of x shifted by tap (di, dj) for chunk ch: [C, CH, W]."""
    if dj == 1:
        xsrc, coff = xbfB, 2
    else:
        xsrc, coff = xbfA, dj
    r0 = ch * CH + di
    return xsrc[:, r0 : r0 + CH, coff : coff + W]


def _build_program():
    import concourse.tile as tile
    import concourse.mybir as mybir
    from concourse import bacc

    fp32 = mybir.dt.float32
    bf16 = mybir.dt.bfloat16
    AF = mybir.ActivationFunctionType

    nc = bacc.Bacc("TRN2", target_bir_lowering=False, debug=False, num_devices=8)
    xs_d = nc.dram_tensor("xs", [C, HC + 2, W], fp32, kind="ExternalInput")
    lhsT_d = nc.dram_tensor("lhsT", [C, 9 * 72], bf16, kind="ExternalInput")
    w4T_d = nc.dram_tensor("w4T", [C, C], bf16, kind="ExternalInput")
    R3_d = nc.dram_tensor("R3", [72, 3 * 80], bf16, kind="ExternalInput")
    idm_d = nc.dram_tensor("idm", [C, C], bf16, kind="ExternalInput")
    bias_d = nc.dram_tensor("bias72", [72, 1], fp32, kind="ExternalInput")
    out_d = nc.dram_tensor("out", [C, HC, W], fp32, kind="ExternalOutput")

    with tile.TileContext(nc) as tc:
        with (
            tc.tile_pool(name="const", bufs=1) as cp,
            tc.tile_pool(name="work", bufs=1) as wp,
            tc.tile_pool(name="tmp", bufs=3) as tp,
            tc.tile_pool(name="psT", bufs=2, space="PSUM") as ppT,
            tc.tile_pool(name="psP", bufs=2, space="PSUM") as ppP,
            tc.tile_pool(name="psS", bufs=3, space="PSUM") as ppS,
            tc.tile_pool(name="psO", bufs=1, space="PSUM") as ppO,
        ):
            lhsT = cp.tile([C, 9 * 72], bf16)
            nc.sync.dma_start(lhsT[:], lhsT_d[:])
            w4T = cp.tile([C, C], bf16)
            nc.sync.dma_start(w4T[:], w4T_d[:])
            R3 = cp.tile([72, 3 * 80], bf16)
            nc.sync.dma_start(R3[:], R3_d[:])
            idm = cp.tile([C, C], bf16)
            nc.sync.dma_start(idm[:], idm_d[:])
            bias = cp.tile([72, 1], fp32)
            nc.sync.dma_start(bias[:], bias_d[:])

            # all activation funcs used (Abs, Ln, Exp, Copy) live in table 6
            # (natural_log_exp_and_others); pin it once so the act-table pass
            # doesn't ping-pong between exp_and_others and natural_log.
            nc.scalar.add_instruction(
                mybir.InstLoadActFuncSet(
                    name=nc.get_next_instruction_name(),
                    act_func_set_id=6,
                    ins=[],
                    outs=[],
                )
            )

            # input: fp32 via HWDGE, then engine casts to the two bf16 tiles
            # (the SWDGE cast-DMA path costs ~12us of Q7 drain at the head)
            RH = 9  # rows needed for chunk 0 taps
            xsf = cp.tile([C, HC + 2, W], fp32)
            nc.sync.dma_start(xsf[:, 0:RH, :], xs_d[:, 0:RH, :])
            nc.sync.dma_start(xsf[:, RH:, :], xs_d[:, RH:, :])
            xbfA = cp.tile([C, HC + 2, RA], bf16)
            xbfB = cp.tile([C, HC + 2, RB], bf16)
            nc.vector.memset(xbfA[:, :, 0:1], 0)
            nc.vector.memset(xbfA[:, :, 57:58], 0)
            nc.vector.memset(xbfB[:, :, 0:2], 0)
            nc.vector.memset(xbfB[:, :, 58:60], 0)
            nc.scalar.copy(xbfA[:, 0:RH, 1:57], xsf[:, 0:RH, :])
            nc.scalar.copy(xbfA[:, RH:, 1:57], xsf[:, RH:, :])
            nc.vector.tensor_copy(out=xbfB[:, :, 2:58], in_=xsf[:])

            # ---- phase 1: T matmuls + phi (log space) for all chunks ----
            lnphis = []
            for ch in range(NCH):
                psT = ppT.tile([72, CHN], fp32, tag="psT")
                for tap in range(9):
                    di, dj = tap // 3, tap % 3
                    rhs = xbfA[:, ch * CH + di : ch * CH + di + CH, dj : dj + W]
                    nc.tensor.matmul(
                        psT[:],
                        lhsT[:, tap * 72 : (tap + 1) * 72],
                        rhs,
                        start=(tap == 0),
                        stop=(tap == 8),
                    )
                u = tp.tile([72, CHN], fp32, tag="u", bufs=2)
                nc.scalar.activation(u[:], psT[:], AF.Abs, bias=bias[:], scale=1.0)
                m = tp.tile([72, CHN], fp32, tag="m", bufs=2)
                nc.vector.tensor_scalar_min(m[:], u[:], 1.0 - 1e-6)
                lnphi = tp.tile([72, CHN], bf16, tag="lnphi", bufs=4)
                nc.scalar.activation(lnphi[:], m[:], AF.Ln, bias=1.0, scale=-1.0)
                lnphis.append(lnphi)

            # ---- phase 2: replication, samp, MAC, k-sum, store ----
            for ch in range(NCH):
                lnphi = lnphis[ch]
                prods = []
                for di in range(3):
                    # ---- replication matmul + exp -> Phi patterns ----
                    psP = ppP.tile([80, CHN], fp32, tag="psP")
                    nc.tensor.matmul(
                        psP[:],
                        R3[:, di * 80 : (di + 1) * 80],
                        lnphi[:],
                        start=True,
                        stop=True,
                    )
                    Ph = tp.tile([80, CHN], bf16, tag="Ph")
                    nc.scalar.activation(Ph[:], psP[:], AF.Exp)

                    # ---- samp matmuls (row-group packed) + drain/mul ----
                    prod = tp.tile([C, 3, CHN], bf16, tag="prod")
                    sdrain = (ch, di) in SDRAIN
                    psSs = []
                    for dj in range(3):
                        psS = ppS.tile([C, CHN], fp32, tag="psS")
                        nc.tensor.matmul(
                            psS[:],
                            w4T[32 * dj : 32 * dj + 16, :],
                            Ph[32 * dj : 32 * dj + 16, :],
                            start=True,
                            stop=True,
                            tile_position=(32 * dj, 0),
                        )
                        psSs.append(psS)
                    for dj in range(3):
                        xv = _x_window(xbfA, xbfB, ch, di, dj)
                        pv = prod[:, dj, :].rearrange("p (h w) -> p h w", h=CH)
                        sv = psSs[dj][:].rearrange("p (h w) -> p h w", h=CH)
                        if sdrain:
                            dr = tp.tile([C, CHN], bf16, tag="dr")
                            nc.scalar.copy(dr[:], psSs[dj][:])
                            nc.vector.tensor_mul(
                                pv, dr[:].rearrange("p (h w) -> p h w", h=CH), xv
                            )
                        else:
                            nc.vector.tensor_mul(pv, sv, xv)
                    prods.append(prod)

                # ---- k-sum ----
                res = tp.tile([C, CHN], fp32, tag="res")
                if KSUM == "pe":
                    psO = ppO.tile([C, CHN], fp32, tag="psO")
                    for di in range(3):
                        for dj in range(3):
                            nc.tensor.matmul(
                                psO[:],
                                idm[:],
                                prods[di][:, dj, :],
                                start=(di == 0 and dj == 0),
                                stop=(di == 2 and dj == 2),
                            )
                    nc.scalar.copy(res[:], psO[:])
                else:
                    t1 = tp.tile([C, 3, CHN], bf16, tag="t1")
                    nc.vector.tensor_add(t1[:], prods[0][:], prods[1][:])
                    nc.vector.tensor_add(t1[:], t1[:], prods[2][:])
                    t2 = tp.tile([C, CHN], bf16, tag="t2")
                    nc.vector.tensor_add(t2[:], t1[:, 0, :], t1[:, 1, :])
                    nc.vector.tensor_add(res[:], t2[:], t1[:, 2, :])
                nc.sync.dma_start(
                    out_d[:, ch * CH : (ch + 1) * CH, :],
                    res[:].rearrange("p (h w) -> p h w", h=CH),
                )

    nc.finalize()
    return nc


def _prep_inputs(x, offset_w, offset_b, weight):
    """Host-side sharding + weight reshaping. Returns per-core input maps."""
    x = np.asarray(x, dtype=np.float32)
    offset_w = np.asarray(offset_w, dtype=np.float32)
    offset_b = np.asarray(offset_b, dtype=np.float32)
    weight = np.asarray(weight, dtype=np.float32)

    # lhsT[c, tap*72 + k*8 + axis*4 + i] = offset_w[2k+axis, c, tap//3, tap%3]
    ow = offset_w.reshape(9, 2, C, 3, 3)  # [k, axis, c, di, dj]
    lhsT = np.transpose(ow, (2, 3, 4, 0, 1))  # [c, di, dj, k, axis]
    lhsT = np.repeat(lhsT[..., None], 4, axis=-1)  # [c, di, dj, k, axis, i]
    lhsT = np.ascontiguousarray(lhsT.reshape(C, 648)).astype(_BF16)

    # w4T rows 32g+s = weight[:, s//4, s%4]
    w4T = np.zeros((C, C), dtype=_BF16)
    wT = weight.reshape(C, 16).T.astype(_BF16)  # [16, C]
    for g in range(4):
        w4T[32 * g : 32 * g + 16, :] = wT

    # R3[8k+4a+i, 80*di + 32*dj + 4*yi + xi]: log-space replication matrices
    R3 = np.zeros((72, 240), dtype=_BF16)
    for di in range(3):
        for dj in range(3):
            k = 3 * di + dj
            for yi in range(4):
                for xi in range(4):
                    col = 80 * di + 32 * dj + 4 * yi + xi
                    R3[8 * k + yi, col] = 1
                    R3[8 * k + 4 + xi, col] = 1

    idm = np.eye(C, dtype=_BF16)

    # bias72[k*8+axis*4+i] = offset_b[2k+axis] + base - i
    base = np.arange(3, dtype=np.float32) + 0.5
    bias = np.zeros((9, 2, 4), dtype=np.float32)
    for k in range(9):
        for axis in range(2):
            bv = base[k // 3] if axis == 0 else base[k % 3]
            bias[k, axis, :] = offset_b[2 * k + axis] + bv - np.arange(4)
    bias72 = bias.reshape(72, 1)

    in_maps = []
    for core in range(8):
        b, half = core // 2, core % 2
        h0 = half * HC
        xs = np.zeros((C, HC + 2, W), dtype=np.float32)
        lo, hi = h0 - 1, h0 + HC + 1
        slo, shi = max(lo, 0), min(hi, H)
        xs[:, slo - lo : slo - lo + (shi - slo), :] = x[b, :, slo:shi, :]
        in_maps.append(
            {
                "xs": xs,
                "lhsT": lhsT,
                "w4T": w4T,
                "R3": R3,
                "idm": idm,
                "bias72": bias72,
            }
        )
    return in_maps


def kernel(x, offset_w, offset_b, weight):
    from concourse.bass_utils import run_bass_kernel_spmd

    if "nc" not in _cache:
        _cache["nc"] = _build_program()
    nc = _cache["nc"]

    in_maps = _prep_inputs(x, offset_w, offset_b, weight)
    res = run_bass_kernel_spmd(nc, in_maps, core_ids=list(range(8)))

    out = np.zeros((B, C, H, W), dtype=np.float32)
    for core in range(8):
        b, half = core // 2, core % 2
        out[b, :, half * HC : (half + 1) * HC, :] = res.results[core]["out"].reshape(
            C, HC, W
        )
    return out


# revision 5
# speedup vs baseline: 1.8571x; 1.0091x over previous
"""Trainium2 Bass kernel for nn_DeformKernelConv2d (v3: log-space replication).

Math (per batch image; shapes below are per core after sharding):
  offsets:  off = conv3x3(x, offset_w) + offset_b          -> dy,dx per (k, pixel)
  coords:   yc_k = dy_k + by_k ; xc_k = dx_k + bx_k        (scope-kernel space)
  phi:      phi_y[k,i] = relu(1-|yc_k - i|), i=0..3        (likewise phi_x)
  Phi:      Phi_k[4*yi+xi] = phi_y[k,yi] * phi_x[k,xi]     (bilinear weights, 16 per k)
  samp:     samp_k[c] = sum_s wflat[c,s] * Phi_k[s]        (matmul over s=16)
  out:      out[c] = sum_k samp_k[c] * x_k[c]              (x_k = 3x3-shifted x)

v3 pipeline (all per core; 4 chunks of 7 rows = 392 px):
  phase 1 (per chunk): 9-tap T matmul -> psT[72,392]; u=Abs(psT+bias) [S];
    m=min(u,1-eps) [V]; lnphi=Ln(1-m) [S]  (= ln(relu(1-u)+eps), bf16).
  phase 2 (per chunk, k grouped 3x3 as k=3*di+dj):
    - replication matmuls: psP = R_di^T @ lnphi; partition 32*dj+4*yi+xi
      accumulates ln(phi_y)+ln(phi_x) for k=3*di+dj (log-space product);
      di=0,1 share a 2-bank psP tile so one Exp covers both.
    - Ph = Exp(psP) [S] -> bf16 bilinear weights Phi.
    - samp matmuls per (di,dj): wT (16 rows at base 32*dj) @ Ph slice,
      row-group packed -> psS fp32.
    - prod = psS * x_shift(di,dj): V fused from PSUM, or S drain + V bf16
      mul (SDRAIN set balances the two engines).
    - k-sum: bf16 add tree; big pair-adds on V, small folds on GpSimd,
      final fp32 add on V -> res -> DMA out.
"""

import numpy as np
import ml_dtypes

B, C, H, W = 4, 128, 56, 56
HC = H // 2            # 28 rows per core
NPIX = HC * W          # 1568
CH = 7                 # chunk height (rows)
NCH = HC // CH         # 4 chunks
CHN = CH * W           # 392 columns per chunk
RA, RB = 58, 60        # padded row lengths: xbfA data at col 1, xbfB at col 2
PSB = 512              # psum bank size in fp32 elems

# packed bf16 const layout: [lhsT 648 | w4T 128 | R3 240]
PK_LHST, PK_W4T, PK_R3 = 0, 648, 648 + 128
PKW = 648 + 128 + 240

# (chunk, di) groups whose samp drain goes scalar-drain + bf16-mul path
SDRAIN = {(0, 0), (1, 0), (2, 0), (3, 0), (1, 1), (3, 1)}

_BF16 = ml_dtypes.bfloat16
_cache = {}


def _x_window(xbfA, xbfB, ch, di, dj):
    """bf16 view of x shifted by tap (di, dj) for chunk ch: [C, CH, W]."""
    if dj == 1:
        xsrc, coff = xbfB, 2
    else:
        xsrc, coff = xbfA, dj
    r0 = ch * CH + di
    return xsrc[:, r0 : r0 + CH, coff : coff + W]


def _build_program():
    import concourse.tile as tile
    import concourse.mybir as mybir
    from concourse import bacc

    fp32 = mybir.dt.float32
    bf16 = mybir.dt.bfloat16
    AF = mybir.ActivationFunctionType

    nc = bacc.Bacc("TRN2", target_bir_lowering=False, debug=False, num_devices=8)
    xs_d = nc.dram_tensor("xs", [C, HC + 2, W], fp32, kind="ExternalInput")
    pk_d = nc.dram_tensor("pk", [C, PKW], bf16, kind="ExternalInput")
    bias_d = nc.dram_tensor("bias72", [72, 1], fp32, kind="ExternalInput")
    out_d = nc.dram_tensor("out", [C, HC, W], fp32, kind="ExternalOutput")

    with tile.TileContext(nc) as tc:
        with (
            tc.tile_pool(name="const", bufs=1) as cp,
            tc.tile_pool(name="tmp", bufs=3) as tp,
            tc.tile_pool(name="psT", bufs=2, space="PSUM") as ppT,
            tc.tile_pool(name="psP", bufs=1, space="PSUM") as ppP,
            tc.tile_pool(name="psS", bufs=3, space="PSUM") as ppS,
        ):
            RH = 9  # rows needed for chunk 0 taps
            xsf = cp.tile([C, HC + 2, W], fp32)
            nc.sync.dma_start(xsf[:, 0:RH, :], xs_d[:, 0:RH, :])
            pk = cp.tile([C, PKW], bf16)
            nc.sync.dma_start(pk[:], pk_d[:])
            bias = cp.tile([72, 1], fp32)
            nc.sync.dma_start(bias[:], bias_d[:])
            nc.sync.dma_start(xsf[:, RH:, :], xs_d[:, RH:, :])
            lhsT = pk[:, PK_LHST : PK_LHST + 648]
            w4T = pk[:, PK_W4T : PK_W4T + 128]
            R3 = pk[0:72, PK_R3 : PK_R3 + 240]

            # all activation funcs used (Abs, Ln, Exp, Copy) live in table 6
            # (natural_log_exp_and_others); pin it once so the act-table pass
            # doesn't ping-pong between exp_and_others and natural_log.
            nc.scalar.add_instruction(
                mybir.InstLoadActFuncSet(
                    name=nc.get_next_instruction_name(),
                    act_func_set_id=6,
                    ins=[],
                    outs=[],
                )
            )

            # input cast to the two bf16 tiles on Scalar/Vector (the SWDGE
            # cast-DMA path costs ~12us of Q7 drain at the head)
            xbfA = cp.tile([C, HC + 2, RA], bf16)
            xbfB = cp.tile([C, HC + 2, RB], bf16)
            nc.vector.memset(xbfA[:, :, 0:1], 0)
            nc.vector.memset(xbfA[:, :, 57:58], 0)
            nc.vector.memset(xbfB[:, :, 0:2], 0)
            nc.vector.memset(xbfB[:, :, 58:60], 0)
            nc.scalar.copy(xbfA[:, 0:RH, 1:57], xsf[:, 0:RH, :])
            nc.scalar.copy(xbfA[:, RH:, 1:57], xsf[:, RH:, :])
            nc.vector.tensor_copy(out=xbfB[:, :, 2:58], in_=xsf[:])

            # ---- phase 1: T matmuls + phi (log space) for all chunks ----
            lnphis = []
            for ch in range(NCH):
                psT = ppT.tile([72, CHN], fp32, tag="psT")
                for tap in range(9):
                    di, dj = tap // 3, tap % 3
                    rhs = xbfA[:, ch * CH + di : ch * CH + di + CH, dj : dj + W]
                    nc.tensor.matmul(
                        psT[:],
                        lhsT[:, tap * 72 : (tap + 1) * 72],
                        rhs,
                        start=(tap == 0),
                        stop=(tap == 8),
                    )
                u = tp.tile([72, CHN], fp32, tag="u", bufs=2)
                nc.scalar.activation(u[:], psT[:], AF.Abs, bias=bias[:], scale=1.0)
                m = tp.tile([72, CHN], fp32, tag="m", bufs=2)
                nc.vector.tensor_scalar_min(m[:], u[:], 1.0 - 1e-6)
                lnphi = tp.tile([72, CHN], bf16, tag="lnphi", bufs=4)
                nc.scalar.activation(lnphi[:], m[:], AF.Ln, bias=1.0, scale=-1.0)
                lnphis.append(lnphi)

            # ---- phase 2: replication, samp, MAC, k-sum, store ----
            for ch in range(NCH):
                lnphi = lnphis[ch]
                # replication matmuls; di=0,1 share a 2-bank tile -> one Exp
                psP01 = ppP.tile([80, 2, PSB], fp32, tag="psP01")
                psP2 = ppP.tile([80, PSB], fp32, tag="psP2")
                for di in range(3):
                    dst = psP01[:, di, 0:CHN] if di < 2 else psP2[:, 0:CHN]
                    nc.tensor.matmul(
                        dst,
                        R3[:, di * 80 : (di + 1) * 80],
                        lnphi[:],
                        start=True,
                        stop=True,
                    )
                Ph = tp.tile([80, 3, CHN], bf16, tag="Ph", bufs=2)
                nc.scalar.activation(Ph[:, 0:2, :], psP01[:, :, 0:CHN], AF.Exp)
                nc.scalar.activation(Ph[:, 2, :], psP2[:, 0:CHN], AF.Exp)

                prods = []
                for di in range(3):
                    prod = tp.tile([C, 3, CHN], bf16, tag="prod", bufs=4)
                    sdrain = (ch, di) in SDRAIN
                    psSs = []
                    for dj in range(3):
                        psS = ppS.tile([C, CHN], fp32, tag="psS")
                        nc.tensor.matmul(
                            psS[:],
                            w4T[32 * dj : 32 * dj + 16, :],
                            Ph[32 * dj : 32 * dj + 16, di, :],
                            start=True,
                            stop=True,
                            tile_position=(32 * dj, 0),
                        )
                        psSs.append(psS)
                    for dj in range(3):
                        xv = _x_window(xbfA, xbfB, ch, di, dj)
                        pv = prod[:, dj, :].rearrange("p (h w) -> p h w", h=CH)
                        sv = psSs[dj][:].rearrange("p (h w) -> p h w", h=CH)
                        if sdrain:
                            dr = tp.tile([C, CHN], bf16, tag="dr", bufs=3)
                            nc.scalar.copy(dr[:], psSs[dj][:])
                            nc.vector.tensor_mul(
                                pv, dr[:].rearrange("p (h w) -> p h w", h=CH), xv
                            )
                        else:
                            nc.vector.tensor_mul(pv, sv, xv)
                    prods.append(prod)

                # ---- k-sum: V big adds, GpSimd folds, V final fp32 add ----
                t1 = tp.tile([C, 3, CHN], bf16, tag="t1", bufs=2)
                nc.vector.tensor_add(t1[:], prods[0][:], prods[1][:])
                nc.vector.tensor_add(t1[:], t1[:], prods[2][:])
                t2 = tp.tile([C, CHN], bf16, tag="t2", bufs=2)
                nc.gpsimd.tensor_add(t2[:], t1[:, 0, :], t1[:, 1, :])
                res = tp.tile([C, CHN], fp32, tag="res", bufs=2)
                nc.vector.tensor_add(res[:], t2[:], t1[:, 2, :])
                nc.sync.dma_start(
                    out_d[:, ch * CH : (ch + 1) * CH, :],
                    res[:].rearrange("p (h w) -> p h w", h=CH),
                )

    nc.finalize()
    return nc


def _prep_inputs(x, offset_w, offset_b, weight):
    """Host-side sharding + weight reshaping. Returns per-core input maps."""
    x = np.asarray(x, dtype=np.float32)
    offset_w = np.asarray(offset_w, dtype=np.float32)
    offset_b = np.asarray(offset_b, dtype=np.float32)
    weight = np.asarray(weight, dtype=np.float32)

    # lhsT[c, tap*72 + k*8 + axis*4 + i] = offset_w[2k+axis, c, tap//3, tap%3]
    ow = offset_w.reshape(9, 2, C, 3, 3)  # [k, axis, c, di, dj]
    lhsT = np.transpose(ow, (2, 3, 4, 0, 1))  # [c, di, dj, k, axis]
    lhsT = np.repeat(lhsT[..., None], 4, axis=-1)  # [c, di, dj, k, axis, i]
    lhsT = lhsT.reshape(C, 648)

    # w4T rows 32g+s = weight[:, s//4, s%4]
    w4T = np.zeros((C, C), dtype=np.float32)
    wT = weight.reshape(C, 16).T  # [16, C]
    for g in range(4):
        w4T[32 * g : 32 * g + 16, :] = wT

    # R3[8k+4a+i, 80*di + 32*dj + 4*yi + xi]: log-space replication matrices
    R3 = np.zeros((C, 240), dtype=np.float32)
    for di in range(3):
        for dj in range(3):
            k = 3 * di + dj
            for yi in range(4):
                for xi in range(4):
                    col = 80 * di + 32 * dj + 4 * yi + xi
                    R3[8 * k + yi, col] = 1
                    R3[8 * k + 4 + xi, col] = 1

    pk = np.concatenate([lhsT, w4T, R3], axis=1).astype(_BF16)

    # bias72[k*8+axis*4+i] = offset_b[2k+axis] + base - i
    base = np.arange(3, dtype=np.float32) + 0.5
    bias = np.zeros((9, 2, 4), dtype=np.float32)
    for k in range(9):
        for axis in range(2):
            bv = base[k // 3] if axis == 0 else base[k % 3]
            bias[k, axis, :] = offset_b[2 * k + axis] + bv - np.arange(4)
    bias72 = bias.reshape(72, 1)

    in_maps = []
    for core in range(8):
        b, half = core // 2, core % 2
        h0 = half * HC
        xs = np.zeros((C, HC + 2, W), dtype=np.float32)
        lo, hi = h0 - 1, h0 + HC + 1
        slo, shi = max(lo, 0), min(hi, H)
        xs[:, slo - lo : slo - lo + (shi - slo), :] = x[b, :, slo:shi, :]
        in_maps.append({"xs": xs, "pk": pk, "bias72": bias72})
    return in_maps


def kernel(x, offset_w, offset_b, weight):
    from concourse.bass_utils import run_bass_kernel_spmd

    if "nc" not in _cache:
        _cache["nc"] = _build_program()
    nc = _cache["nc"]

    in_maps = _prep_inputs(x, offset_w, offset_b, weight)
    res = run_bass_kernel_spmd(nc, in_maps, core_ids=list(range(8)))

    out = np.zeros((B, C, H, W), dtype=np.float32)
    for core in range(8):
        b, half = core // 2, core % 2
        out[b, :, half * HC : (half + 1) * HC, :] = res.results[core]["out"].reshape(
            C, HC, W
        )
    return out


# revision 8
# speedup vs baseline: 1.9805x; 1.0664x over previous
"""Trainium2 Bass kernel for nn_DeformKernelConv2d (v3: log-space replication).

Math (per batch image; shapes below are per core after sharding):
  offsets:  off = conv3x3(x, offset_w) + offset_b          -> dy,dx per (k, pixel)
  coords:   yc_k = dy_k + by_k ; xc_k = dx_k + bx_k        (scope-kernel space)
  phi:      phi_y[k,i] = relu(1-|yc_k - i|), i=0..3        (likewise phi_x)
  Phi:      Phi_k[4*yi+xi] = phi_y[k,yi] * phi_x[k,xi]     (bilinear weights, 16 per k)
  samp:     samp_k[c] = sum_s wflat[c,s] * Phi_k[s]        (matmul over s=16)
  out:      out[c] = sum_k samp_k[c] * x_k[c]              (x_k = 3x3-shifted x)

v3 pipeline (all per core; 4 chunks of 7 rows = 392 px):
  phase 1 (per chunk): 9-tap T matmul -> psT[72,392]; u=Abs(psT+bias) [S];
    m=min(u,1-eps) [V]; lnphi=Ln(1-m) [S]  (= ln(relu(1-u)+eps), bf16).
  phase 2 (per chunk, k grouped 3x3 as k=3*di+dj):
    - replication matmuls: psP = R_di^T @ lnphi; partition 32*dj+4*yi+xi
      accumulates ln(phi_y)+ln(phi_x) for k=3*di+dj (log-space product);
      di=0,1 share a 2-bank psP tile so one Exp covers both.
    - Ph = Exp(psP) [S] -> bf16 bilinear weights Phi.
    - samp matmuls per (di,dj): wT (16 rows at base 32*dj) @ Ph slice,
      row-group packed -> psS fp32.
    - prod = psS * x_shift(di,dj): V fused from PSUM, or S drain + V bf16
      mul (SDRAIN set balances the two engines).
    - k-sum: bf16 add tree; big pair-adds on V, small folds on GpSimd,
      final fp32 add on V -> res -> DMA out.
"""

import numpy as np
import ml_dtypes

B, C, H, W = 4, 128, 56, 56
HC = H // 2            # 28 rows per core
NPIX = HC * W          # 1568
CH = 7                 # chunk height (rows)
NCH = HC // CH         # 4 chunks
CHN = CH * W           # 392 columns per chunk
RA, RB = 58, 60        # padded row lengths: xbfA data at col 1, xbfB at col 2
PSB = 512              # psum bank size in fp32 elems

# packed bf16 const layout: [lhsT 648 | w4T 128 | R3 240]
PK_LHST, PK_W4T, PK_R3 = 0, 648, 648 + 128
PKW = 648 + 128 + 240

# (chunk, di) groups whose samp drain goes scalar-drain + bf16-mul path
SDRAIN = {(0, 0), (1, 0), (2, 0), (3, 0), (1, 1), (3, 1)}

_BF16 = ml_dtypes.bfloat16
_cache = {}


def _x_window(xbfA, xbfB, ch, di, dj):
    """bf16 view of x shifted by tap (di, dj) for chunk ch: [C, CH, W]."""
    if dj == 1:
        xsrc, coff = xbfB, 2
    else:
        xsrc, coff = xbfA, dj
    r0 = ch * CH + di
    return xsrc[:, r0 : r0 + CH, coff : coff + W]


def _build_program():
    import concourse.tile as tile
    import concourse.mybir as mybir
    from concourse import bacc

    fp32 = mybir.dt.float32
    bf16 = mybir.dt.bfloat16
    AF = mybir.ActivationFunctionType

    nc = bacc.Bacc("TRN2", target_bir_lowering=False, debug=False, num_devices=8)
    xa_d = nc.dram_tensor("xa", [C, HC + 2, RA], bf16, kind="ExternalInput")
    xb_d = nc.dram_tensor("xb", [C, HC + 2, RB], bf16, kind="ExternalInput")
    pk_d = nc.dram_tensor("pk", [C, PKW], bf16, kind="ExternalInput")
    bias_d = nc.dram_tensor("bias72", [72, 1], fp32, kind="ExternalInput")
    out_d = nc.dram_tensor("out", [C, HC, W], fp32, kind="ExternalOutput")

    with tile.TileContext(nc) as tc:
        with (
            tc.tile_pool(name="const", bufs=1) as cp,
            tc.tile_pool(name="tmp", bufs=3) as tp,
            tc.tile_pool(name="psT", bufs=2, space="PSUM") as ppT,
            tc.tile_pool(name="psP", bufs=1, space="PSUM") as ppP,
            tc.tile_pool(name="psS", bufs=3, space="PSUM") as ppS,
        ):
            pk = cp.tile([C, PKW], bf16)
            nc.sync.dma_start(pk[:], pk_d[:])
            xbfA = cp.tile([C, HC + 2, RA], bf16)
            nc.sync.dma_start(xbfA[:], xa_d[:])
            bias = cp.tile([72, 1], fp32)
            nc.sync.dma_start(bias[:], bias_d[:])
            xbfB = cp.tile([C, HC + 2, RB], bf16)
            nc.sync.dma_start(xbfB[:], xb_d[:])
            lhsT = pk[:, PK_LHST : PK_LHST + 648]
            w4T = pk[:, PK_W4T : PK_W4T + 128]
            R3 = pk[0:72, PK_R3 : PK_R3 + 240]

            # all activation funcs used (Abs, Ln, Exp, Copy) live in table 6
            # (natural_log_exp_and_others); pin it once so the act-table pass
            # doesn't ping-pong between exp_and_others and natural_log.
            nc.scalar.add_instruction(
                mybir.InstLoadActFuncSet(
                    name=nc.get_next_instruction_name(),
                    act_func_set_id=6,
                    ins=[],
                    outs=[],
                )
            )

            # ---- phase 1: T matmuls + phi (log space) for all chunks ----
            lnphis = []
            for ch in range(NCH):
                psT = ppT.tile([72, CHN], fp32, tag="psT")
                for tap in range(9):
                    di, dj = tap // 3, tap % 3
                    rhs = xbfA[:, ch * CH + di : ch * CH + di + CH, dj : dj + W]
                    nc.tensor.matmul(
                        psT[:],
                        lhsT[:, tap * 72 : (tap + 1) * 72],
                        rhs,
                        start=(tap == 0),
                        stop=(tap == 8),
                    )
                u = tp.tile([72, CHN], fp32, tag="u", bufs=2)
                nc.scalar.activation(u[:], psT[:], AF.Abs, bias=bias[:], scale=1.0)
                m = tp.tile([72, CHN], fp32, tag="m", bufs=2)
                nc.vector.tensor_scalar_min(m[:], u[:], 1.0 - 1e-6)
                lnphi = tp.tile([72, CHN], bf16, tag="lnphi", bufs=4)
                nc.scalar.activation(lnphi[:], m[:], AF.Ln, bias=1.0, scale=-1.0)
                lnphis.append(lnphi)

            # ---- phase 2: replication, samp, MAC, k-sum, store ----
            for ch in range(NCH):
                lnphi = lnphis[ch]
                # replication matmuls; di=0,1 share a 2-bank tile -> one Exp
                psP01 = ppP.tile([80, 2, PSB], fp32, tag="psP01")
                psP2 = ppP.tile([80, PSB], fp32, tag="psP2")
                for di in range(3):
                    dst = psP01[:, di, 0:CHN] if di < 2 else psP2[:, 0:CHN]
                    nc.tensor.matmul(
                        dst,
                        R3[:, di * 80 : (di + 1) * 80],
                        lnphi[:],
                        start=True,
                        stop=True,
                    )
                Ph = tp.tile([80, 3, CHN], bf16, tag="Ph", bufs=2)
                nc.scalar.activation(Ph[:, 0:2, :], psP01[:, :, 0:CHN], AF.Exp)
                nc.scalar.activation(Ph[:, 2, :], psP2[:, 0:CHN], AF.Exp)

                prods = []
                for di in range(3):
                    prod = tp.tile([C, 3, CHN], bf16, tag="prod", bufs=4)
                    sdrain = (ch, di) in SDRAIN
                    psSs = []
                    for dj in range(3):
                        psS = ppS.tile([C, CHN], fp32, tag="psS")
                        nc.tensor.matmul(
                            psS[:],
                            w4T[32 * dj : 32 * dj + 16, :],
                            Ph[32 * dj : 32 * dj + 16, di, :],
                            start=True,
                            stop=True,
                            tile_position=(32 * dj, 0),
                        )
                        psSs.append(psS)
                    for dj in range(3):
                        xv = _x_window(xbfA, xbfB, ch, di, dj)
                        pv = prod[:, dj, :].rearrange("p (h w) -> p h w", h=CH)
                        sv = psSs[dj][:].rearrange("p (h w) -> p h w", h=CH)
                        if sdrain:
                            dr = tp.tile([C, CHN], bf16, tag="dr", bufs=3)
                            nc.scalar.copy(dr[:], psSs[dj][:])
                            nc.vector.tensor_mul(
                                pv, dr[:].rearrange("p (h w) -> p h w", h=CH), xv
                            )
                        else:
                            nc.vector.tensor_mul(pv, sv, xv)
                    prods.append(prod)

                # ---- k-sum: bf16 add tree on V ----
                t1 = tp.tile([C, 3, CHN], bf16, tag="t1", bufs=2)
                nc.vector.tensor_add(t1[:], prods[0][:], prods[1][:])
                nc.vector.tensor_add(t1[:], t1[:], prods[2][:])
                t2 = tp.tile([C, CHN], bf16, tag="t2", bufs=2)
                nc.vector.tensor_add(t2[:], t1[:, 0, :], t1[:, 1, :])
                res = tp.tile([C, CHN], fp32, tag="res", bufs=2)
                nc.vector.tensor_add(res[:], t2[:], t1[:, 2, :])
                nc.sync.dma_start(
                    out_d[:, ch * CH : (ch + 1) * CH, :],
                    res[:].rearrange("p (h w) -> p h w", h=CH),
                )

    nc.finalize()
    return nc


def _prep_inputs(x, offset_w, offset_b, weight):
    """Host-side sharding + weight reshaping. Returns per-core input maps."""
    x = np.asarray(x, dtype=np.float32)
    offset_w = np.asarray(offset_w, dtype=np.float32)
    offset_b = np.asarray(offset_b, dtype=np.float32)
    weight = np.asarray(weight, dtype=np.float32)

    # lhsT[c, tap*72 + k*8 + axis*4 + i] = offset_w[2k+axis, c, tap//3, tap%3]
    ow = offset_w.reshape(9, 2, C, 3, 3)  # [k, axis, c, di, dj]
    lhsT = np.transpose(ow, (2, 3, 4, 0, 1))  # [c, di, dj, k, axis]
    lhsT = np.repeat(lhsT[..., None], 4, axis=-1)  # [c, di, dj, k, axis, i]
    lhsT = lhsT.reshape(C, 648)

    # w4T rows 32g+s = weight[:, s//4, s%4]
    w4T = np.zeros((C, C), dtype=np.float32)
    wT = weight.reshape(C, 16).T  # [16, C]
    for g in range(4):
        w4T[32 * g : 32 * g + 16, :] = wT

    # R3[8k+4a+i, 80*di + 32*dj + 4*yi + xi]: log-space replication matrices
    R3 = np.zeros((C, 240), dtype=np.float32)
    for di in range(3):
        for dj in range(3):
            k = 3 * di + dj
            for yi in range(4):
                for xi in range(4):
                    col = 80 * di + 32 * dj + 4 * yi + xi
                    R3[8 * k + yi, col] = 1
                    R3[8 * k + 4 + xi, col] = 1

    pk = np.concatenate([lhsT, w4T, R3], axis=1).astype(_BF16)

    # bias72[k*8+axis*4+i] = offset_b[2k+axis] + base - i
    base = np.arange(3, dtype=np.float32) + 0.5
    bias = np.zeros((9, 2, 4), dtype=np.float32)
    for k in range(9):
        for axis in range(2):
            bv = base[k // 3] if axis == 0 else base[k % 3]
            bias[k, axis, :] = offset_b[2 * k + axis] + bv - np.arange(4)
    bias72 = bias.reshape(72, 1)

    in_maps = []
    for core in range(8):
        b, half = core // 2, core % 2
        h0 = half * HC
        xs = np.zeros((C, HC + 2, W), dtype=np.float32)
        lo, hi = h0 - 1, h0 + HC + 1
        slo, shi = max(lo, 0), min(hi, H)
        xs[:, slo - lo : slo - lo + (shi - slo), :] = x[b, :, slo:shi, :]
        xsb = xs.astype(_BF16)
        xa = np.zeros((C, HC + 2, RA), dtype=_BF16)
        xa[:, :, 1:57] = xsb
        xb = np.zeros((C, HC + 2, RB), dtype=_BF16)
        xb[:, :, 2:58] = xsb
        in_maps.append({"xa": xa, "xb": xb, "pk": pk, "bias72": bias72})
    return in_maps


def kernel(x, offset_w, offset_b, weight):
    from concourse.bass_utils import run_bass_kernel_spmd

    if "nc" not in _cache:
        _cache["nc"] = _build_program()
    nc = _cache["nc"]

    in_maps = _prep_inputs(x, offset_w, offset_b, weight)
    res = run_bass_kernel_spmd(nc, in_maps, core_ids=list(range(8)))

    out = np.zeros((B, C, H, W), dtype=np.float32)
    for core in range(8):
        b, half = core // 2, core % 2
        out[b, :, half * HC : (half + 1) * HC, :] = res.results[core]["out"].reshape(
            C, HC, W
        )
    return out
